# revision 17
# baseline (speedup 1.0000x reference)
"""AttentionCopyDecoder on 8 TRN2 NeuronCores.

Sharding: batch (B=64 -> 8/core) for the attention/GRU/copy-attention half,
vocab (V=50000 -> 6250/core) for the output projection + softmax + scatter.
Cross-core: one AllGather (decoder vectors + scatter weights), one AllReduce
(softmax denominator). The copy-mechanism scatter-add runs as 5 GPSIMD
local_scatter ops over pre-grouped duplicate totals.
"""
import os
import numpy as np
import ml_dtypes

B, T, H2, E, V = 64, 400, 1024, 512, 50000
NC = 8
BS = B // NC          # 8 batch rows per core
VS = V // NC          # 6250 vocab cols per core
EPS = 1e-8
TCH = [128, 128, 128, 16]   # t-chunks (400)
NVT = 13                    # vocab tiles of <=512 (12*512 + 106)
VTS = [512] * 12 + [106]
NE = 1250                   # local_scatter chunk width (5 * 1250 = 6250)

_STATE = {}
LAST_RESULTS = None


def _build():
    import concourse.bass as bass
    import concourse.bacc as bacc
    import concourse.mybir as mybir
    import concourse.tile as tile
    from concourse.masks import make_identity
    from concourse import library_config

    dt = mybir.dt
    nc = bacc.Bacc("TRN2", target_bir_lowering=False, debug=False,
                   enable_asserts=False, num_devices=NC)

    def inp(name, shape, dtype=dt.float32):
        return nc.dram_tensor(name, shape, dtype, kind="ExternalInput").ap()

    def outp(name, shape, dtype=dt.float32):
        return nc.dram_tensor(name, shape, dtype, kind="ExternalOutput").ap()

    enc_nat = inp("enc_nat", [BS, T, H2])
    enc_t = inp("enc_t", [BS, H2, T], dt.float32r)
    pre_hT = inp("pre_hT", [H2, BS])
    embT = inp("embT", [E, BS])
    idsF = inp("idsF", [BS, 1])
    srcF_b = inp("srcF_b", [BS, T])
    src_all = inp("src_all", [B, T])
    vb64 = inp("vb64", [B, 1])
    a1T = inp("a1T", [H2, 1], dt.float32r)
    a2T = inp("a2T", [H2, 1])
    ab = inp("ab", [1, 1])
    dnn_Wt = inp("dnn_Wt", [E + H2, E])
    dnn_bT = inp("dnn_bT", [128, E // 128])
    comb_Wt = inp("comb_Wt", [H2 + E, E])
    comb_bT = inp("comb_bT", [128, E // 128])
    gru_WihT = inp("gru_WihT", [E, 3 * H2])
    gru_WhhT = inp("gru_WhhT", [H2, 3 * H2])
    gru_bihT = inp("gru_bihT", [128, 24])
    gru_bhhT = inp("gru_bhhT", [128, 24])
    copy_Wt = inp("copy_Wt", [H2, H2], dt.float32r)
    copy_bT = inp("copy_bT", [128, H2 // 128])
    outWt = inp("outWt", [2560, VS], dt.bfloat16)
    out_bT = inp("out_bT", [1, VS], dt.float32r)

    out_shard = outp("out_shard", [B, VS])
    h_new_o = outp("h_new", [BS, H2])
    aw_o = outp("aw_out", [BS, T])
    caw_o = outp("caw_out", [BS, T])

    f32, f32r, bf16 = dt.float32, dt.float32r, dt.bfloat16
    i16 = dt.int16
    AF = mybir.ActivationFunctionType
    OP = mybir.AluOpType
    X = mybir.AxisListType.X

    with tile.TileContext(nc) as tc:
        with tc.tile_pool(name="const", bufs=1) as cst, \
             tc.tile_pool(name="dram", bufs=1, space="DRAM") as dram:
            nc.gpsimd.load_library(library_config.local_scatter)
            ident = cst.tile([128, 128], f32)
            make_identity(nc, ident[:])
            ones64f = cst.tile([1, B], f32)
            nc.vector.memset(ones64f[:], 1.0)
            ones64 = cst.tile([1, B], f32r)
            nc.vector.tensor_copy(ones64[:], ones64f[:])

            a1_sb = cst.tile([128, 8], f32r)
            nc.sync.dma_start(a1_sb[:], a1T.rearrange("(c p) o -> p (c o)", p=128))
            a2_sb = cst.tile([128, 8], f32)
            nc.sync.dma_start(a2_sb[:], a2T.rearrange("(c p) o -> p (c o)", p=128))
            ab_sb = cst.tile([1, 1], f32)
            nc.sync.dma_start(ab_sb[:], ab[:])
            preh_sb = cst.tile([128, 8, BS], f32)
            nc.sync.dma_start(preh_sb[:], pre_hT.rearrange("(c p) b -> p c b", p=128))
            emb_sb = cst.tile([128, 4, BS], f32)
            nc.sync.dma_start(emb_sb[:], embT.rearrange("(c p) b -> p c b", p=128))
            ids_sb = cst.tile([BS, 1], f32)
            nc.sync.dma_start(ids_sb[:], idsF[:])
            srcb_sb = cst.tile([BS, T], f32)
            nc.sync.dma_start(srcb_sb[:], srcF_b[:])
            dnnb_sb = cst.tile([128, 4], f32)
            nc.sync.dma_start(dnnb_sb[:], dnn_bT[:])
            combb_sb = cst.tile([128, 4], f32)
            nc.sync.dma_start(combb_sb[:], comb_bT[:])
            bih_sb = cst.tile([128, 24], f32)
            nc.sync.dma_start(bih_sb[:], gru_bihT[:])
            bhh_sb = cst.tile([128, 24], f32)
            nc.sync.dma_start(bhh_sb[:], gru_bhhT[:])
            copyb_sb = cst.tile([128, 8], f32)
            nc.sync.dma_start(copyb_sb[:], copy_bT[:])

            # collective bounce buffers
            GSZ = 128 * 192      # dT block (160 cols) + w' block (32 cols)
            g_in = dram.tile([GSZ], f32)
            g_out = dram.tile([NC * GSZ], f32, addr_space="Shared")
            ar_in = dram.tile([1, B], f32)
            ar_out = dram.tile([1, B], f32, addr_space="Shared")
            w_dram = dram.tile([B * T], f32)

            # ============ phase A: batch-sharded half ============
            with tc.tile_pool(name="pa", bufs=1) as pa:
                awlog = pa.tile([BS, T], f32, tag="awlog")
                awlog_f = pa.tile([1, BS * T], f32, tag="awlogf")
                crt = pa.tile([128, 2, 8, BS], f32, tag="crt")

                with tc.tile_pool(name="ps1", bufs=2, space="PSUM") as ps1:
                    # s_b = pre_h . a2 + align_b  -> [1, BS]
                    s_ps = ps1.tile([1, BS], f32, tag="sps")
                    for c in range(8):
                        nc.tensor.matmul(s_ps[:], a2_sb[:, c:c + 1], preh_sb[:, c, :],
                                         start=(c == 0), stop=(c == 7))
                    s_sb = pa.tile([1, BS], f32, tag="s")
                    nc.vector.tensor_scalar_add(s_sb[:], s_ps[:], ab_sb[:, 0:1])

                    for b in range(BS):
                        encT = pa.tile([128, 8, T], f32r, tag="encT")
                        nc.sync.dma_start(
                            encT[:], enc_t[b].rearrange("(c p) t -> p c t", p=128))
                        awl_ps = ps1.tile([1, T], f32, tag="awl")
                        for c in range(8):
                            nc.tensor.matmul(awl_ps[:], a1_sb[:, c:c + 1],
                                             encT[:, c, :],
                                             start=(c == 0), stop=(c == 7))
                        nc.scalar.activation(awlog_f[:, b * T:(b + 1) * T],
                                             awl_ps[:], AF.Tanh,
                                             bias=s_sb[:, b:b + 1])
                    nc.sync.dma_start(awlog[:], awlog_f[:])

                # aw = softmax(awlog) over T
                expw = pa.tile([BS, T], f32, tag="expw")
                den = pa.tile([BS, 1], f32, tag="den")
                nc.scalar.activation(expw[:], awlog[:], AF.Exp, accum_out=den[:])
                rden = pa.tile([BS, 1], f32, tag="rden")
                nc.vector.reciprocal(rden[:], den[:])
                aw_sb = pa.tile([BS, T], f32, tag="aw")
                nc.vector.tensor_scalar_mul(aw_sb[:], expw[:], rden[:])
                nc.sync.dma_start(aw_o[:], aw_sb[:])

                # masked renormalized weights
                mask = pa.tile([BS, T], f32, tag="mask")
                nc.vector.tensor_scalar(mask[:], srcb_sb[:], ids_sb[:, 0:1], None,
                                        op0=OP.is_equal)
                w_sb = pa.tile([BS, T], f32, tag="w")
                nc.vector.tensor_mul(w_sb[:], aw_sb[:], mask[:])
                wsum = pa.tile([BS, 1], f32, tag="wsum")
                nc.vector.reduce_sum(wsum[:], w_sb[:], axis=X)
                nc.vector.tensor_scalar_add(wsum[:], wsum[:], EPS)
                rw = pa.tile([BS, 1], f32, tag="rw")
                nc.vector.reciprocal(rw[:], wsum[:])
                wn_sb = pa.tile([BS, T], f32, tag="wn")
                nc.vector.tensor_scalar_mul(wn_sb[:], w_sb[:], rw[:])

                # W2T[t, c, col]: cols 0..7 = aw per b, 32..39 = wn per b
                w2in = pa.tile([64, T], f32, tag="w2in")
                nc.vector.memset(w2in[:], 0.0)
                nc.vector.tensor_copy(w2in[0:BS, :], aw_sb[:])
                nc.vector.tensor_copy(w2in[32:32 + BS, :], wn_sb[:])
                w2T = pa.tile([128, 4, 64], f32, tag="w2T")
                with tc.tile_pool(name="ps2", bufs=2, space="PSUM") as ps2:
                    for c, tc_n in enumerate(TCH):
                        tp = ps2.tile([128, 64], f32, tag="w2tp")
                        nc.tensor.transpose(tp[:tc_n, :],
                                            w2in[:, c * 128:c * 128 + tc_n],
                                            ident[:64, :64])
                        nc.vector.tensor_copy(w2T[:tc_n, c, :], tp[:tc_n, :])

                # context/readM, transposed into crt[k, {ctx,rdM}, kc, b]
                with tc.tile_pool(name="ps3", bufs=3, space="PSUM") as ps3:
                    for b in range(BS):
                        encN = pa.tile([128, 4, H2], f32, tag="encN")
                        nc.sync.dma_start(
                            encN[:, 0:3, :],
                            enc_nat[b][0:384].rearrange("(c p) k -> p c k", p=128))
                        nc.sync.dma_start(encN[0:16, 3, :], enc_nat[b][384:400])
                        for kc in range(8):
                            cr_ps = ps3.tile([128, 2], f32, tag="crps")
                            for c, tc_n in enumerate(TCH):
                                nc.tensor.matmul(
                                    cr_ps[:],
                                    encN[:tc_n, c, kc * 128:(kc + 1) * 128],
                                    w2T[:tc_n, c, b::32],
                                    start=(c == 0), stop=(c == 3))
                            nc.vector.tensor_copy(crt[:, :, kc, b], cr_ps[:])

                # ---- batched MLPs + GRU (transposed, free dim = BS) ----
                emb2T = pa.tile([128, 4, BS], f32, tag="emb2T")
                niT = pa.tile([128, 4, BS], f32, tag="niT")
                hnewT = pa.tile([128, 8, BS], f32, tag="hnewT")
                cawlog = pa.tile([BS, T], f32, tag="cawlog")
                cawlog_f = pa.tile([1, BS * T], f32, tag="cawlogf")
                with tc.tile_pool(name="pw", bufs=1) as pw:
                    bigw = pw.tile([128, 12288], f32, tag="bigw")
                    copyw = pw.tile([128, 8, H2], f32r, tag="copyw")
                    nc.sync.dma_start(
                        copyw[:], copy_Wt.rearrange("(c p) o -> p c o", p=128))
                    with tc.tile_pool(name="ps4", bufs=4, space="PSUM") as ps4:
                        dnnw = bigw[:, :12 * E].rearrange("p (c o) -> p c o", c=12)
                        nc.sync.dma_start(
                            dnnw, dnn_Wt.rearrange("(c p) o -> p c o", p=128))
                        for oc in range(4):
                            ps = ps4.tile([128, BS], f32, tag="mlp")
                            for fc in range(12):
                                rhs = (emb_sb[:, fc, :] if fc < 4
                                       else crt[:, 1, fc - 4, :])
                                nc.tensor.matmul(
                                    ps[:], dnnw[:, fc, oc * 128:(oc + 1) * 128],
                                    rhs, start=(fc == 0), stop=(fc == 11))
                            nc.scalar.activation(emb2T[:, oc, :], ps[:], AF.Identity,
                                                 bias=dnnb_sb[:, oc:oc + 1])

                        combw = bigw[:, :12 * E].rearrange("p (c o) -> p c o", c=12)
                        nc.sync.dma_start(
                            combw, comb_Wt.rearrange("(c p) o -> p c o", p=128))
                        for oc in range(4):
                            ps = ps4.tile([128, BS], f32, tag="mlp")
                            for fc in range(12):
                                rhs = (emb2T[:, fc, :] if fc < 4
                                       else crt[:, 0, fc - 4, :])
                                nc.tensor.matmul(
                                    ps[:], combw[:, fc, oc * 128:(oc + 1) * 128],
                                    rhs, start=(fc == 0), stop=(fc == 11))
                            nc.scalar.activation(niT[:, oc, :], ps[:], AF.Relu,
                                                 bias=combb_sb[:, oc:oc + 1])

                        giT = pa.tile([128, 24, BS], f32, tag="giT")
                        wih = bigw[:, :4 * 3 * H2].rearrange("p (c o) -> p c o", c=4)
                        nc.sync.dma_start(
                            wih, gru_WihT.rearrange("(c p) o -> p c o", p=128))
                        for oc in range(24):
                            ps = ps4.tile([128, BS], f32, tag="mlp")
                            for fc in range(4):
                                nc.tensor.matmul(
                                    ps[:], wih[:, fc, oc * 128:(oc + 1) * 128],
                                    niT[:, fc, :], start=(fc == 0), stop=(fc == 3))
                            nc.scalar.activation(giT[:, oc, :], ps[:], AF.Identity,
                                                 bias=bih_sb[:, oc:oc + 1])
                        ghT = pa.tile([128, 24, BS], f32, tag="ghT")
                        for half in range(2):
                            whh = bigw[:, :4 * 3 * H2].rearrange(
                                "p (c o) -> p c o", c=4)
                            nc.sync.dma_start(
                                whh, gru_WhhT[half * 512:(half + 1) * 512]
                                .rearrange("(c p) o -> p c o", p=128))
                            for oc in range(24):
                                ps = ps4.tile([128, BS], f32, tag="mlp")
                                for fc in range(4):
                                    nc.tensor.matmul(
                                        ps[:], whh[:, fc, oc * 128:(oc + 1) * 128],
                                        preh_sb[:, half * 4 + fc, :],
                                        start=(fc == 0), stop=(fc == 3))
                                if half == 0:
                                    nc.scalar.activation(
                                        ghT[:, oc, :], ps[:], AF.Identity,
                                        bias=bhh_sb[:, oc:oc + 1])
                                else:
                                    nc.vector.tensor_add(ghT[:, oc, :],
                                                         ghT[:, oc, :], ps[:])

                        tmp1 = pa.tile([128, BS], f32, tag="tmp1")
                        tmp2 = pa.tile([128, BS], f32, tag="tmp2")
                        for oc in range(8):
                            nc.vector.tensor_add(tmp1[:], giT[:, oc, :], ghT[:, oc, :])
                            rT = pa.tile([128, BS], f32, tag="rT")
                            nc.scalar.activation(rT[:], tmp1[:], AF.Sigmoid)
                            nc.vector.tensor_add(tmp1[:], giT[:, 8 + oc, :],
                                                 ghT[:, 8 + oc, :])
                            zT = pa.tile([128, BS], f32, tag="zT")
                            nc.scalar.activation(zT[:], tmp1[:], AF.Sigmoid)
                            nc.vector.tensor_mul(tmp1[:], rT[:], ghT[:, 16 + oc, :])
                            nc.vector.tensor_add(tmp1[:], tmp1[:], giT[:, 16 + oc, :])
                            nT = pa.tile([128, BS], f32, tag="nT")
                            nc.scalar.activation(nT[:], tmp1[:], AF.Tanh)
                            nc.vector.tensor_sub(tmp2[:], preh_sb[:, oc, :], nT[:])
                            nc.vector.tensor_mul(tmp2[:], zT[:], tmp2[:])
                            nc.vector.tensor_add(hnewT[:, oc, :], nT[:], tmp2[:])

                        hn_sb = pa.tile([BS, H2], f32, tag="hn")
                        for kc in range(8):
                            tp = ps4.tile([BS, 128], f32, tag="hntp")
                            nc.tensor.transpose(tp[:], hnewT[:, kc, :], ident[:])
                            nc.vector.tensor_copy(hn_sb[:, kc * 128:(kc + 1) * 128],
                                                  tp[:])
                        nc.sync.dma_start(h_new_o[:], hn_sb[:])

                    # ---- copy attention: ca = sigmoid(enc @ copyW.T + b) ----
                    with tc.tile_pool(name="ps5", bufs=2, space="PSUM") as ps5:
                        hnr = pa.tile([128, 8, BS], f32r, tag="hnr")
                        nc.vector.tensor_copy(hnr[:], hnewT[:])
                        for b in range(BS):
                            encT = pa.tile([128, 8, T], f32r, tag="encT")
                            nc.sync.dma_start(
                                encT[:],
                                enc_t[b].rearrange("(c p) t -> p c t", p=128))
                            cl_ps = ps5.tile([1, T], f32, tag="clps")
                            for dc in range(8):
                                ca_ps = ps5.tile([128, T], f32, tag="caps")
                                for kc in range(8):
                                    nc.tensor.matmul(
                                        ca_ps[:],
                                        copyw[:, kc, dc * 128:(dc + 1) * 128],
                                        encT[:, kc, :],
                                        start=(kc == 0), stop=(kc == 7))
                                caT = pa.tile([128, T], f32r, tag="caT")
                                nc.scalar.activation(caT[:], ca_ps[:], AF.Sigmoid,
                                                     bias=copyb_sb[:, dc:dc + 1])
                                nc.tensor.matmul(cl_ps[:], hnr[:, dc, b:b + 1],
                                                 caT[:],
                                                 start=(dc == 0), stop=(dc == 7))
                            nc.scalar.copy(cawlog_f[:, b * T:(b + 1) * T],
                                           cl_ps[:])
                        nc.sync.dma_start(cawlog[:], cawlog_f[:])

                expc = pa.tile([BS, T], f32, tag="expc")
                denc = pa.tile([BS, 1], f32, tag="denc")
                nc.scalar.activation(expc[:], cawlog[:], AF.Exp, accum_out=denc[:])
                rdenc = pa.tile([BS, 1], f32, tag="rdenc")
                nc.vector.reciprocal(rdenc[:], denc[:])
                caw_sb = pa.tile([BS, T], f32, tag="cawsb")
                nc.vector.tensor_scalar_mul(caw_sb[:], expc[:], rdenc[:])
                nc.sync.dma_start(caw_o[:], caw_sb[:])

                # ---- group-total copy weights w' per b (for the scatter) ----
                wT = pa.tile([128, 4, BS], f32, tag="wT")
                nc.vector.memset(wT[:], 0.0)
                with tc.tile_pool(name="ps6", bufs=2, space="PSUM") as ps6:
                    srcT = pa.tile([128, 4, BS], f32, tag="srcT")
                    srcin = pa.tile([64, T], f32, tag="srcin")
                    nc.vector.memset(srcin[:], 0.0)
                    nc.vector.tensor_copy(srcin[0:BS, :], srcb_sb[:])
                    for c, tc_n in enumerate(TCH):
                        tp = ps6.tile([128, 64], f32, tag="srctp")
                        nc.tensor.transpose(tp[:tc_n, :],
                                            srcin[:, c * 128:c * 128 + tc_n],
                                            ident[:64, :64])
                        nc.vector.tensor_copy(srcT[:tc_n, c, :], tp[:tc_n, 0:BS])
                    for b in range(BS):
                        D_sb = pa.tile([128, 4, T], f32, tag="D")
                        for c, tc_n in enumerate(TCH):
                            # bc[j, t] = src[b, c*128+t] for all j: transpose of
                            # a free-broadcast column (exact value movement)
                            bc_ps = ps6.tile([128, 128], f32, tag="bcps")
                            nc.tensor.transpose(
                                bc_ps[:, :tc_n],
                                srcT[:tc_n, c, b:b + 1].to_broadcast(
                                    [tc_n, 128]),
                                ident[:tc_n, :tc_n])
                            for cp, tcp in enumerate(TCH):
                                nc.vector.tensor_scalar(
                                    D_sb[:tcp, cp, c * 128:c * 128 + tc_n],
                                    bc_ps[:tcp, :tc_n],
                                    srcT[:tcp, cp, b:b + 1], None,
                                    op0=OP.is_equal)
                        for c, tc_n in enumerate(TCH):
                            wp_ps = ps6.tile([128, 1], f32, tag="wpps")
                            for cp, tcp in enumerate(TCH):
                                nc.tensor.matmul(
                                    wp_ps[:tc_n, :],
                                    D_sb[:tcp, cp, c * 128:c * 128 + tc_n],
                                    w2T[:tcp, cp, b:b + 1],
                                    start=(cp == 0), stop=(cp == 3))
                            nc.vector.tensor_copy(wT[:tc_n, c, b:b + 1],
                                                  wp_ps[:tc_n, :])

                dT_reg = g_in[0:GSZ].rearrange("(p f) -> p f", p=128)
                nc.sync.dma_start(dT_reg[:, 0:32],
                                  emb2T[:].rearrange("p c b -> p (c b)"))
                nc.sync.dma_start(dT_reg[:, 32:96],
                                  crt[:, 0].rearrange("p c b -> p (c b)"))
                nc.sync.dma_start(dT_reg[:, 96:160],
                                  hnewT[:].rearrange("p c b -> p (c b)"))
                nc.sync.dma_start(dT_reg[:, 160:192],
                                  wT[:].rearrange("p c b -> p (c b)"))

            nc.gpsimd.collective_compute(
                "AllGather", OP.bypass, replica_groups=[list(range(NC))],
                ins=[g_in[:]], outs=[g_out[:]])

            # ============ phase B: vocab-sharded half ============
            with tc.tile_pool(name="pb", bufs=1) as pb, \
                 tc.tile_pool(name="psB", bufs=3, space="PSUM") as psB:
                dT_all = pb.tile([128, 20, B], f32, tag="dTall")
                w_gath = pb.tile([128, NC, 32], f32, tag="wgath")
                for r in range(NC):
                    tmp = pb.tile([128, 192], f32, tag="gtmp")
                    nc.sync.dma_start(
                        tmp[:], g_out[r * GSZ:(r + 1) * GSZ]
                        .rearrange("(p f) -> p f", p=128))
                    nc.vector.tensor_copy(
                        dT_all[:, :, r * BS:(r + 1) * BS],
                        tmp[:, 0:160].rearrange("p (c b) -> p c b", c=20))
                    nc.vector.tensor_copy(w_gath[:, r, :], tmp[:, 160:192])
                dT_bf = pb.tile([128, 20, B], bf16, tag="dTbf")
                nc.vector.tensor_copy(dT_bf[:], dT_all[:])

                # w' -> [64, 400] via PE transpose + per-partition-contiguous DMA
                wtr = pb.tile([128, 2, 128], f32, tag="wtr")
                for h in range(2):
                    tp = psB.tile([128, 128], f32, tag="wtrp")
                    nc.tensor.transpose(
                        tp[:], w_gath[:].rearrange("p r k -> p (r k)")
                        [:, h * 128:(h + 1) * 128], ident[:])
                    nc.vector.tensor_copy(wtr[:, h, :], tp[:])
                wv = w_dram[:].rearrange("(b t) -> b t", b=B)
                for r in range(NC):
                    base = (r % 4) * 32
                    half = r // 4
                    # rows r*32+c*8+b hold w'[b, c*128 + p]; c<3 full width
                    nc.sync.dma_start(
                        w_dram[r * BS * T:(r + 1) * BS * T]
                        .rearrange("(b ct) -> b ct", b=BS)[:, 0:384]
                        .rearrange("b (c p) -> c b p", c=3),
                        wtr[base:base + 24, half, :])
                    nc.sync.dma_start(
                        wv[r * BS:(r + 1) * BS, 384:400],
                        wtr[base + 24:base + 32, half, 0:16])
                w_all = pb.tile([B, T], f32, tag="wall")
                nc.sync.dma_start(w_all[:], wv[:])
                w_bf = pb.tile([B, T], bf16, tag="wbf")
                nc.vector.tensor_copy(w_bf[:], w_all[:])

                outb_sb = pb.tile([1, VS], f32r, tag="outb")
                nc.sync.dma_start(outb_sb[:], out_bT[:])

                exp_sb = pb.tile([B, NVT, 512], f32, tag="exp")
                part = pb.tile([B, NVT], f32, tag="part")
                wtv = outWt.rearrange("(c p) v -> p c v", p=128)
                for vt in range(NVT):
                    n = VTS[vt]
                    lg = psB.tile([B, 512], f32, tag="lg")
                    wt_sb = pb.tile([128, 20, 512], bf16, tag="wt")
                    nc.sync.dma_start(wt_sb[:, :, :n],
                                      wtv[:, :, vt * 512:vt * 512 + n])
                    for fc in range(20):
                        nc.tensor.matmul(lg[:, :n], dT_bf[:, fc, :],
                                         wt_sb[:, fc, :n],
                                         start=(fc == 0), stop=False)
                    nc.tensor.matmul(lg[:, :n], ones64[:],
                                     outb_sb[:, vt * 512:vt * 512 + n],
                                     start=False, stop=True)
                    nc.scalar.activation(exp_sb[:, vt, :n], lg[:, :n], AF.Exp,
                                         accum_out=part[:, vt:vt + 1])

                tot = pb.tile([B, 1], f32, tag="tot")
                nc.vector.reduce_sum(tot[:], part[:], axis=X)
                nc.sync.dma_start(ar_in[:].rearrange("o b -> b o"), tot[:])
                nc.gpsimd.collective_compute(
                    "AllReduce", OP.add, replica_groups=[list(range(NC))],
                    ins=[ar_in[:]], outs=[ar_out[:]])
                totg = pb.tile([B, 1], f32, tag="totg")
                nc.sync.dma_start(totg[:], ar_out[:].rearrange("o b -> b o"))
                rsum = pb.tile([B, 1], f32, tag="rsum")
                nc.vector.reciprocal(rsum[:], totg[:])

                # local_scatter chunk indices from src_all
                srcall_sb = pb.tile([B, T], f32, tag="srcall")
                nc.sync.dma_start(srcall_sb[:], src_all[:])
                vb_sb = pb.tile([B, 1], f32, tag="vb")
                nc.sync.dma_start(vb_sb[:], vb64[:])
                local = pb.tile([B, T], f32, tag="local")
                nc.vector.tensor_scalar(local[:], srcall_sb[:], vb_sb[:, 0:1],
                                        None, op0=OP.subtract)

                gen_full = pb.tile([B, NVT * 512], f32, tag="genf")
                for vt in range(NVT):
                    n = VTS[vt]
                    nc.vector.tensor_scalar_mul(
                        gen_full[:, vt * 512:vt * 512 + n],
                        exp_sb[:, vt, :n], rsum[:, 0:1])

                for k in range(5):
                    t1 = pb.tile([B, T], f32, tag="t1")
                    nc.vector.tensor_scalar(t1[:], local[:], float(k * NE), None,
                                            op0=OP.subtract)
                    m1 = pb.tile([B, T], f32, tag="m1")
                    nc.vector.tensor_scalar(m1[:], t1[:], 0.0, None, op0=OP.is_ge)
                    m2 = pb.tile([B, T], f32, tag="m2")
                    nc.vector.tensor_scalar(m2[:], t1[:], float(NE), None,
                                            op0=OP.is_lt)
                    nc.vector.tensor_mul(m1[:], m1[:], m2[:])
                    # idx = m ? t1 : -1  ==  (t1 + 1) * m - 1
                    nc.vector.tensor_scalar_add(t1[:], t1[:], 1.0)
                    nc.vector.tensor_mul(t1[:], t1[:], m1[:])
                    nc.vector.tensor_scalar_add(t1[:], t1[:], -1.0)
                    idx16 = pb.tile([B, T], i16, tag="idx16")
                    nc.vector.tensor_copy(idx16[:], t1[:])
                    sppb = pb.tile([B, NE], bf16, tag=f"sppb{k}")
                    nc.gpsimd.local_scatter(sppb[:], w_bf[:], idx16[:],
                                            channels=B, num_elems=NE,
                                            num_idxs=T)
                    spf = pb.tile([B, NE], f32, tag="spf")
                    nc.vector.tensor_copy(spf[:], sppb[:])
                    nc.vector.tensor_add(gen_full[:, k * NE:(k + 1) * NE],
                                         gen_full[:, k * NE:(k + 1) * NE],
                                         spf[:])

                nc.sync.dma_start(out_shard[:], gen_full[:, 0:VS])

    nc.compile()
    return nc


def kernel(**inputs):
    global LAST_RESULTS
    import concourse.bass_utils as bass_utils

    if "nc" not in _STATE:
        _STATE["nc"] = _build()
    nc = _STATE["nc"]

    f32 = np.float32
    g = {k: np.asarray(v) for k, v in inputs.items()}
    enc = np.ascontiguousarray(g["enc_out"], dtype=f32)
    ids = g["input_ids"].astype(np.int64)            # (B,1)
    src = g["source_input"]                          # (B,T)
    emb_rows = np.ascontiguousarray(
        g["emb_table"], dtype=f32)[ids[:, 0]]        # (B,E)
    preh = g["pre_hidden"][:, 0, :].astype(f32)      # (B,H2)

    a1T = np.ascontiguousarray(g["align_W"][0, :H2, None], dtype=f32)
    a2T = np.ascontiguousarray(g["align_W"][0, H2:, None], dtype=f32)
    ab = g["align_b"].reshape(1, 1).astype(f32)
    dnn_Wt = np.ascontiguousarray(g["dnn_W"].T, dtype=f32)
    dnn_bT = np.ascontiguousarray(g["dnn_b"].reshape(4, 128).T, dtype=f32)
    comb_Wt = np.ascontiguousarray(g["comb_W"].T, dtype=f32)
    comb_bT = np.ascontiguousarray(g["comb_b"].reshape(4, 128).T, dtype=f32)
    gru_WihT = np.ascontiguousarray(g["gru_Wih"].T, dtype=f32)
    gru_WhhT = np.ascontiguousarray(g["gru_Whh"].T, dtype=f32)
    bihT = np.ascontiguousarray(g["gru_bih"].reshape(24, 128).T, dtype=f32)
    bhhT = np.ascontiguousarray(g["gru_bhh"].reshape(24, 128).T, dtype=f32)
    copy_Wt = np.ascontiguousarray(g["copy_W"].T, dtype=f32)
    copy_bT = np.ascontiguousarray(g["copy_b"].reshape(8, 128).T, dtype=f32)
    outWt = np.ascontiguousarray(g["out_W"].T, dtype=f32)    # (2560, V)
    outWt_bf = outWt.astype(ml_dtypes.bfloat16)
    out_b = g["out_b"].astype(f32)
    srcF = src.astype(f32)

    in_maps = []
    for r in range(NC):
        bs = slice(r * BS, (r + 1) * BS)
        vs = slice(r * VS, (r + 1) * VS)
        in_maps.append({
            "enc_nat": enc[bs],
            "enc_t": np.ascontiguousarray(enc[bs].transpose(0, 2, 1)),
            "pre_hT": np.ascontiguousarray(preh[bs].T),
            "embT": np.ascontiguousarray(emb_rows[bs].T),
            "idsF": ids[bs].astype(f32),
            "srcF_b": srcF[bs],
            "src_all": srcF,
            "vb64": np.full((B, 1), r * VS, f32),
            "a1T": a1T, "a2T": a2T, "ab": ab,
            "dnn_Wt": dnn_Wt, "dnn_bT": dnn_bT,
            "comb_Wt": comb_Wt, "comb_bT": comb_bT,
            "gru_WihT": gru_WihT, "gru_WhhT": gru_WhhT,
            "gru_bihT": bihT, "gru_bhhT": bhhT,
            "copy_Wt": copy_Wt, "copy_bT": copy_bT,
            "outWt": np.ascontiguousarray(outWt_bf[:, vs]),
            "out_bT": np.ascontiguousarray(out_b[None, vs]),
        })

    res = bass_utils.run_bass_kernel_spmd(
        nc, in_maps, core_ids=list(range(NC)),
        trace=bool(os.environ.get("KERNEL_TRACE")))
    LAST_RESULTS = res
    R = res.results
    out = np.concatenate([R[r]["out_shard"] for r in range(NC)], axis=1)
    cur_h = np.concatenate([R[r]["h_new"] for r in range(NC)], axis=0)[:, None, :]
    aw = np.concatenate([R[r]["aw_out"] for r in range(NC)], axis=0)[:, None, :]
    caw = np.concatenate([R[r]["caw_out"] for r in range(NC)], axis=0)
    return out, cur_h, aw, caw
